# revision 32
# baseline (speedup 1.0000x reference)
"""Trainium2 Bass kernel for nn_LRSVConv (low-rank spatially-varying conv).

Computes, for full inputs
    x            [8, 32, 256, 256]  f32
    conv_w       [192, 32, 3, 3]    f32   (192 = RANK(3) * C_OUT(64))
    kernel_weight[2, 256, 256]      f32
the reference:
    y   = conv2d(x, conv_w, stride 1, pad 1)      # [8, 192, 256, 256]
    y   = y.reshape(8, 3, 64, 256, 256)
    out = y[:,0] + kw[0]*y[:,1] + kw[1]*y[:,2]    # [8, 64, 256, 256]

Strategy: spatial (H) sharding across 8 cores - each core computes a band of
32 output rows for ALL batches, so the per-pixel blend weights (which are
batch-independent) are loaded once per core and reused 8x.

Final design (v1 baseline measured 150.2-150.5us; this version measures
142.5-145.4us across runs, variance is HAM clock-gate phase luck). The MM
body runs dense at the 216ns/pair-slot warm roofline - the 9-slot/supertile
decomposition's floor is 124.7us and is provably minimal for bf16 direct
conv (ceil(288/128)=3 K-steps x 1.5 M-pairs x 2 px-blocks; fp8 fails the
accuracy gate, tap-baked K=128 layouts exceed the ~245 GB/s measured DMA
fabric). So v2+ attacked the head (was 12.5us to first MM + ~3us cold-clock
penalty) and tail (was ~8us serial fold+DMA after the last MM):
  - bf16 inputs/weights (host-converted; f32 PSUM accumulation).
  - PE column tiling: per supertile (4 rows = 2 blocks q of 512 px),
    9 pair-slots of [96,64,512] matmuls: rank 1 -> ab[:, 0:512], rank 2 ->
    ab[:, 512:1024], rank 0 -> C (last: its bufs=2 bank has a WAR on the
    s-add one supertile back); (q0,q1) adjacent -> concurrent column tiles.
  - kh-baked input layout is premade IN DRAM by the host (xs[b, (kh c),
    band rows]): 1 dma_start per band chunk, no on-chip shuffling. Bands
    load in 4 chunks of 8 rows (dependency granularity: batch b+1's first
    supertiles only dep on chunk 0 - DMA fabric saturates at ~245 GB/s
    aggregate and a full-band transfer arrives too late at batch switch).
  - batch 0's band in 8 per-supertile chunks; c0 gen precedes the wc gen
    (first real MM gates on max of both + ~1.2us DMA completion-receipt
    latency -> first real MM at ~10.3us). 4 warmup matmuls on a zeroed
    tile fill the pre-data window (zero data is invisible to the HAM
    activity monitor - and a CONSTANT nonzero fill measured the whole body
    power-throttled to 2.0 GHz, 173us - so they don't warm the clock;
    starting real MMs ASAP beats idling, cold slots still make progress).
  - fold on the non-PE engines (GPSIMD cannot read PSUM, so psum-reading
    ops live on DVE; DVE op cost ~ free-dim size only):
      DVE:    m = ab * sv (merged [128,1024], bf16 out), s = C + m1
      GPSIMD: out_sb = s + m2 (all-bf16 operands, 1.14us)
    bf16 m/s costs ~1.6e-3 extra rel err (5.8e-3 total vs the 2e-2 gate).
  - output written as [b, t, (q c), (r w)] so each supertile is ONE
    contiguous [128,512] dma_start; descriptor gen on the otherwise-idle
    Scalar engine (gen is ~0.6us serial per issuing engine; Sync's stream
    stays pure input-loads = fabric priority). Host un-shuffles at gather.
  - tail: the last two supertiles split m into m1/m2 (m1 fires 2/3 of a
    supertile early); the final supertile folds in two half-width pieces
    with dma gens on Sync and Scalar in parallel. ~3.3us from last matmul
    to last byte, then a fixed ~9us runtime teardown ceremony.
"""

import os

import numpy as np
from ml_dtypes import bfloat16 as np_bf16

B, C_IN, C_OUT, RANK, IMG = 8, 32, 64, 3, 256
N_CORES = 8
BAND = IMG // N_CORES          # 32 output rows per core
WP = IMG + 2                   # padded width 258
SUPER = 8                      # supertiles per (batch, band): 4 rows each
SROWS = BAND // SUPER          # 4 image rows per supertile
NBLK = 512                     # pixels per matmul block (2 image rows)
# warmup: 2 zero-data queue-filler matmuls + 2 real-data (wc) matmuls

_F32 = np.float32

NB = int(os.environ.get("KERNEL_NB", str(B)))  # batches to process (debug knob)


def _build_bass():
    import concourse.mybir as mybir
    import concourse.tile as tile
    from concourse import bacc

    f32 = mybir.dt.float32
    bf16 = mybir.dt.bfloat16
    nc = bacc.Bacc("TRN2", target_bir_lowering=False, debug=False)

    # xs[b, (kh,cin), (r,w)]: kh-shifted copies premade on host; row r of
    # copy kh is padded-input row (band_start + r + kh), all 258 cols
    xs_t = nc.dram_tensor("xs", (B, 96, BAND * WP), bf16, kind="ExternalInput")
    # wc[(kh,cin), (r,kw), c]: 9 column-tile stationaries of 64 channels
    wc_t = nc.dram_tensor("wc", (96, 9, 64), bf16, kind="ExternalInput")
    # svb[(q,c), t, (s,j)]: per-pixel blend weights for ranks 1 (s=0), 2 (s=1)
    svb_t = nc.dram_tensor("svb", (128, SUPER, 2 * NBLK), bf16, kind="ExternalInput")
    # out[b, t, (q,c), (r,w)]: supertile-contiguous; host unshuffles
    out_t = nc.dram_tensor("out", (B, SUPER, 128, NBLK), bf16, kind="ExternalOutput")

    xs = xs_t.ap()
    out_ap = out_t.ap()

    with tile.TileContext(nc) as tc:
        with (
            tc.tile_pool(name="const", bufs=1) as cpool,
            tc.tile_pool(name="imcol", bufs=3) as ipool,
            tc.tile_pool(name="ps", bufs=3, space="PSUM") as pspool,
            tc.tile_pool(name="tmp", bufs=6) as tpool,
            tc.tile_pool(name="outp", bufs=8) as opool,
        ):
            # --- PE warmup: emitted first so the Tensor queue starts on it
            # at main-start (no DMA deps); releases the HAM clock gate
            # (~3.4us sustained busy) before the first real matmul arrives.
            # Two-phase warmup. Zero-fill data is INVISIBLE to the HAM
            # activity monitor (clock stayed cold through it, every run),
            # and a constant 1.5 fill power-throttled the whole body to
            # 2.0 GHz (173us) - so: 2 zero matmuls just fill the queue
            # until the wc weights land, then matmuls on wc itself
            # (random N(0,0.05) data, statistically like the real body
            # which never throttles) give HAM real toggling ~1.5us before
            # the first input chunk is consumable.
            dummy = cpool.tile([128, NBLK], bf16, tag="dummy")
            nc.vector.memset(dummy[:], 0.0)
            wps = pspool.tile([128, 2 * NBLK], f32, tag="ab", name="wps")
            for _ in range(2):
                nc.tensor.matmul(
                    wps[0:64, 0:NBLK], dummy[0:96, 0:64], dummy[0:96, :],
                    start=True, stop=True,
                )

            # --- DMA gen order = Sync program order; head-critical first.
            wc_sb = cpool.tile([96, 9, 64], bf16)

            # batch 0's band in per-supertile chunks (supertile 0 gated by
            # 198KB instead of 1.6MB)
            b0c = []
            for t in range(SUPER):
                ch = cpool.tile([96, SROWS * WP], bf16, tag=f"b0c{t}", name="ch")
                b0c.append(ch)

            def load_b0(t, eng=None):
                # t=0 goes via GpSimd's SWDGE ring, in parallel with the
                # Sync HWDGE gen stream, so the first chunk lands ~1us
                # earlier and real matmuls start at data-ready
                (eng or nc.sync).dma_start(
                    b0c[t][:], xs[0, :, SROWS * t * WP : SROWS * (t + 1) * WP]
                )

            svb_sb = cpool.tile([128, SUPER, 2 * NBLK], bf16)

            def load_svb(t):
                # per-t loads: one bulk svb DMA jammed the DVE dep chain
                # (m of supertile 2 waited on the whole 1.5MB transfer,
                # backpressured PSUM, 2.5us PE gap + HAM re-throttle)
                nc.sync.dma_start(svb_sb[:, t, :], svb_t.ap()[:, t, :])

            # DMA fabric is ~245 GB/s aggregate (all 16 engines saturate) and
            # service order = gen order, so interleave: each b0 chunk / sv_t
            # lands just ahead of the supertile that reads it.
            # first real MM gates on max(c0-rows01, wc) + ~1.2us receipt
            # each; wc gens on the idle Scalar HWDGE ring so both transfer
            # in parallel, and c0 splits so q0's matmuls gate on 99KB
            nc.scalar.dma_start(wc_sb[:], wc_t.ap())
            nc.sync.dma_start(b0c[0][:, 0 : 2 * WP], xs[0, :, 0 : 2 * WP])
            nc.sync.dma_start(
                b0c[0][:, 2 * WP : SROWS * WP], xs[0, :, 2 * WP : SROWS * WP]
            )

            # real-data warmup on the loaded weights (see above)
            wc_flat = wc_sb.rearrange("p a b -> p (a b)")
            for _ in range(2):
                nc.tensor.matmul(
                    wps[0:64, 0:NBLK], wc_sb[:, 0, :], wc_flat[:, 0:NBLK],
                    start=True, stop=True,
                )
            load_svb(0)
            load_b0(1)
            load_svb(1)
            load_b0(2)
            load_svb(2)
            load_b0(3)
            load_svb(3)
            load_b0(4)
            load_svb(4)
            load_b0(5)
            load_svb(5)
            load_b0(6)
            load_svb(6)
            load_b0(7)
            load_svb(7)

            # batch bands in 4 chunks of 8 rows (2 supertiles each; supertiles
            # never cross a chunk) so batch b+1's first supertiles only dep on
            # chunk 0, not the whole 1.6MB band transfer
            CROWS = BAND // 4

            def load_imcol(b):
                chunks = []
                for j in range(4):
                    ch = ipool.tile(
                        [96, CROWS * WP], bf16, tag="imcol", bufs=8, name="imch"
                    )
                    nc.sync.dma_start(
                        ch[:],
                        xs[b, :, CROWS * j * WP : CROWS * (j + 1) * WP],
                    )
                    chunks.append(ch)
                return chunks

            def emit_conv(iv, hl):
                """18 column-tiled conv matmuls for one supertile."""
                ab = pspool.tile([128, 2 * NBLK], f32, tag="ab", name="ab")
                c = pspool.tile([128, NBLK], f32, tag="c", bufs=2, name="c")
                # rank 1 first (free half 0), then rank 2, then rank 0 -> C:
                # C last keeps slack on its bank's WAR (bufs=2) against the
                # s-add one supertile back. (q0,q1) adjacent -> column-tile
                # pairs on disjoint PSUM partition halves.
                for r, ps in ((1, ab[:, 0:NBLK]), (2, ab[:, NBLK : 2 * NBLK]),
                              (0, c[:, :])):
                    for kw in range(3):
                        st, sp = kw == 0, kw == 2
                        for q in range(2):
                            rhs = iv[:, hl + 2 * q : hl + 2 * q + 2, kw : kw + IMG]
                            o = 64 * q
                            nc.tensor.matmul(
                                ps[o : o + 64, :],
                                wc_sb[:, 3 * r + kw, :], rhs, start=st, stop=sp,
                            )
                return ab, c

            def emit_fold(ab, c, b, t, mode="body"):
                """Blend fold. body: DVE merged-m + s, GPSIMD add, gen on
                Scalar. pen (penultimate): split m on DVE so the DVE FIFO
                drains earlier ahead of the final supertile's chain.
                final: split m on DVE, then two half-width s/add pairs on
                DVE with dma gens on Sync and Scalar in parallel."""
                # m and s in bf16: the psum-reading ops run at f32 rate
                # regardless, but the final add's operands become all-bf16
                # SBUF (DVE/GPSIMD 16-bit fast path) and tmp traffic halves;
                # extra rounding costs ~1e-3 rel err vs the 2e-2 gate
                if mode == "body":
                    # merged multiply keeps DVE busy/supertile low; split
                    # ops measured 1.91us/supertile and backpressured the
                    # PSUM ring into PE gaps
                    m = tpool.tile([128, 2 * NBLK], bf16, tag="m", name="m")
                    nc.vector.tensor_tensor(
                        m[:], ab[:], svb_sb[:, t, :], mybir.AluOpType.mult
                    )
                    m1, m2 = m[:, 0:NBLK], m[:, NBLK : 2 * NBLK]
                else:
                    # tail: split so m1 fires after the r1 matmuls (2/3 of
                    # a supertile early)
                    m1t = tpool.tile([128, NBLK], bf16, tag="m", name="m1t")
                    nc.vector.tensor_tensor(
                        m1t[:], ab[:, 0:NBLK], svb_sb[:, t, 0:NBLK],
                        mybir.AluOpType.mult,
                    )
                    m2t = tpool.tile([128, NBLK], bf16, tag="m2t", name="m2t")
                    nc.vector.tensor_tensor(
                        m2t[:], ab[:, NBLK : 2 * NBLK],
                        svb_sb[:, t, NBLK : 2 * NBLK], mybir.AluOpType.mult,
                    )
                    m1, m2 = m1t[:], m2t[:]
                if mode == "final":
                    H = NBLK // 2
                    for h, deng in enumerate((nc.sync, nc.scalar)):
                        sl = slice(H * h, H * (h + 1))
                        sh = tpool.tile([128, H], bf16, tag=f"sf{h}", name="sh")
                        nc.vector.tensor_tensor(
                            sh[:], c[:, sl], m1[:, sl], mybir.AluOpType.add
                        )
                        oh = opool.tile([128, H], bf16, tag=f"of{h}", name="oh")
                        nc.vector.tensor_tensor(
                            oh[:], sh[:], m2[:, sl], mybir.AluOpType.add
                        )
                        deng.dma_start(out_ap[b, t][:, sl], oh[:])
                    return
                s = tpool.tile([128, NBLK], bf16, tag="s", name="s")
                # s reads PSUM -> must be DVE (GPSIMD cannot access PSUM)
                nc.vector.tensor_tensor(s[:], c[:], m1, mybir.AluOpType.add)
                out_sb = opool.tile([128, NBLK], bf16, tag="out_sb", name="out_sb")
                # out-add always on GPSIMD: putting it on the DVE for the
                # tail measured WORSE (DVE FIFO is the tail bottleneck)
                nc.gpsimd.tensor_tensor(out_sb[:], s[:], m2, mybir.AluOpType.add)
                # descriptor gen on the otherwise-idle Scalar engine: keeps
                # the Sync gen stream pure input-loads (fabric priority)
                nc.scalar.dma_start(out_ap[b, t], out_sb[:])

            imcol = None
            for b in range(NB):
                imcol_nxt = load_imcol(b + 1) if b + 1 < NB else None
                for t in range(SUPER):
                    if b == 0:
                        iv, hl = b0c[t].rearrange("p (h w) -> p h w", w=WP), 0
                    else:
                        iv = imcol[t // 2].rearrange("p (h w) -> p h w", w=WP)
                        hl = SROWS * (t % 2)
                    ab, c = emit_conv(iv, hl)
                    if b == NB - 1 and t == SUPER - 1:
                        mode = "final"
                    elif b == NB - 1 and t == SUPER - 2:
                        mode = "pen"
                    else:
                        mode = "body"
                    emit_fold(ab, c, b, t, mode=mode)
                imcol = imcol_nxt
    nc.compile()
    return nc


_CACHE = {}


def _get_bass():
    if "nc" not in _CACHE:
        _CACHE["nc"] = _build_bass()
    return _CACHE["nc"]


def _prep_shards(x, conv_w, kernel_weight):
    x = np.asarray(x, dtype=_F32)
    conv_w = np.asarray(conv_w, dtype=_F32)
    kernel_weight = np.asarray(kernel_weight, dtype=_F32)

    x_pad = np.pad(x, ((0, 0), (0, 0), (1, 1), (1, 1))).astype(np_bf16)
    # wc[(kh,cin), (r,kw), c] from conv_w[(r c), cin, kh, kw]
    wc = np.ascontiguousarray(
        conv_w.reshape(RANK, C_OUT, C_IN, 3, 3)
        .transpose(3, 2, 0, 4, 1)
        .reshape(96, 9, 64)
    ).astype(np_bf16)

    in_maps = []
    for i in range(N_CORES):
        h0 = BAND * i
        # xs[b, (kh c), (r w)] = x_pad[b, c, h0 + r + kh, w]
        shard = np.ascontiguousarray(
            np.stack(
                [x_pad[:, :, h0 + kh : h0 + kh + BAND, :] for kh in range(3)],
                axis=1,
            )
        ).reshape(B, 96, BAND * WP)
        band = kernel_weight[:, h0 : h0 + BAND, :]          # [2, 32, 256]
        # svb[64q+c, t, (s,j)] = band[s, 4t+2q+(j//256), j%256]
        tmp = band.reshape(2, SUPER, 2, NBLK)               # [s, t, q, j]
        svb = np.broadcast_to(
            tmp.transpose(2, 1, 0, 3)[:, None],             # [q, 1, t, s, j]
            (2, C_OUT, SUPER, 2, NBLK),
        ).reshape(128, SUPER, 2 * NBLK)
        svb = np.ascontiguousarray(svb).astype(np_bf16)
        in_maps.append({"xs": shard, "wc": wc, "svb": svb})
    return in_maps


def run(inputs, trace=False):
    """Run the sharded bass kernel; returns (out_full, BassKernelResults)."""
    from concourse.bass_utils import run_bass_kernel_spmd

    in_maps = _prep_shards(**inputs)
    nc = _get_bass()
    res = run_bass_kernel_spmd(
        nc, in_maps, core_ids=list(range(N_CORES)), trace=trace
    )
    out = np.empty((B, C_OUT, IMG, IMG), dtype=_F32)
    for i in range(N_CORES):
        # res: [B, SUPER, (q c), (r w)] -> [B, c, (t q r), w]
        band = (
            np.asarray(res.results[i]["out"], dtype=_F32)
            .reshape(B, SUPER, 2, C_OUT, 2, IMG)
            .transpose(0, 3, 1, 2, 4, 5)
            .reshape(B, C_OUT, BAND, IMG)
        )
        out[:, :, BAND * i : BAND * (i + 1), :] = band
    return out, res


def kernel(x, conv_w, kernel_weight):
    out, _ = run({"x": x, "conv_w": conv_w, "kernel_weight": kernel_weight})
    return out


# revision 33
# speedup vs baseline: 1.1952x; 1.1952x over previous
"""Trainium2 Bass kernel for nn_LRSVConv (low-rank spatially-varying conv).

Computes, for full inputs
    x            [8, 32, 256, 256]  f32
    conv_w       [192, 32, 3, 3]    f32   (192 = RANK(3) * C_OUT(64))
    kernel_weight[2, 256, 256]      f32
the reference:
    y   = conv2d(x, conv_w, stride 1, pad 1)      # [8, 192, 256, 256]
    y   = y.reshape(8, 3, 64, 256, 256)
    out = y[:,0] + kw[0]*y[:,1] + kw[1]*y[:,2]    # [8, 64, 256, 256]

Strategy: spatial (H) sharding across 8 cores - each core computes a band of
32 output rows for ALL batches, so the per-pixel blend weights (which are
batch-independent) are loaded once per core and reused 8x.

Final design (v1 baseline measured 150.2-150.5us; this version measures
142.5-145.4us across runs, variance is HAM clock-gate phase luck). The MM
body runs dense at the 216ns/pair-slot warm roofline - the 9-slot/supertile
decomposition's floor is 124.7us and is provably minimal for bf16 direct
conv (ceil(288/128)=3 K-steps x 1.5 M-pairs x 2 px-blocks; fp8 fails the
accuracy gate, tap-baked K=128 layouts exceed the ~245 GB/s measured DMA
fabric). So v2+ attacked the head (was 12.5us to first MM + ~3us cold-clock
penalty) and tail (was ~8us serial fold+DMA after the last MM):
  - bf16 inputs/weights (host-converted; f32 PSUM accumulation).
  - PE column tiling: per supertile (4 rows = 2 blocks q of 512 px),
    9 pair-slots of [96,64,512] matmuls: rank 1 -> ab[:, 0:512], rank 2 ->
    ab[:, 512:1024], rank 0 -> C (last: its bufs=2 bank has a WAR on the
    s-add one supertile back); (q0,q1) adjacent -> concurrent column tiles.
  - kh-baked input layout is premade IN DRAM by the host (xs[b, (kh c),
    band rows]): 1 dma_start per band chunk, no on-chip shuffling. Bands
    load in 4 chunks of 8 rows (dependency granularity: batch b+1's first
    supertiles only dep on chunk 0 - DMA fabric saturates at ~245 GB/s
    aggregate and a full-band transfer arrives too late at batch switch).
  - batch 0's band in 8 per-supertile chunks; c0 gen precedes the wc gen
    (first real MM gates on max of both + ~1.2us DMA completion-receipt
    latency -> first real MM at ~10us). Warmup matmuls fill the pre-data
    window: 2 on a zeroed tile (zero data is invisible to the HAM activity
    monitor, but they keep the queue primed), then 2 on the loaded wc
    weights (real random data, so HAM sees toggling ~1.5us before the
    first input chunk is consumable). Starting real MMs ASAP beats idling
    for the clock - cold slots still make progress at half rate.
    NOTE: runs land in occasional ~174us windows where the chip is
    power-throttled to ~2.0 GHz from the very first (zero-data) matmul -
    environmental, not data-dependent.
  - fold on the non-PE engines (GPSIMD cannot read PSUM, so psum-reading
    ops live on DVE; DVE op cost ~ free-dim size only):
      DVE:    m = ab * sv (merged [128,1024], bf16 out), s = C + m1
      GPSIMD: out_sb = s + m2 (all-bf16 operands, 1.14us)
    bf16 m/s costs ~1.6e-3 extra rel err (5.8e-3 total vs the 2e-2 gate).
  - output written as [b, t, (q c), (r w)] so each supertile is ONE
    contiguous [128,512] dma_start; descriptor gen on the otherwise-idle
    Scalar engine (gen is ~0.6us serial per issuing engine; Sync's stream
    stays pure input-loads = fabric priority). Host un-shuffles at gather.
  - tail: the last two supertiles split m into m1/m2 (m1 fires 2/3 of a
    supertile early); the final supertile folds in two half-width pieces
    with dma gens on Sync and Scalar in parallel. ~3.3us from last matmul
    to last byte, then a fixed ~9us runtime teardown ceremony.
"""

import os

import numpy as np
from ml_dtypes import bfloat16 as np_bf16

B, C_IN, C_OUT, RANK, IMG = 8, 32, 64, 3, 256
N_CORES = 8
BAND = IMG // N_CORES          # 32 output rows per core
WP = IMG + 2                   # padded width 258
SUPER = 8                      # supertiles per (batch, band): 4 rows each
SROWS = BAND // SUPER          # 4 image rows per supertile
NBLK = 512                     # pixels per matmul block (2 image rows)
# warmup: 2 zero-data queue-filler matmuls + 2 real-data (wc) matmuls

_F32 = np.float32

NB = int(os.environ.get("KERNEL_NB", str(B)))  # batches to process (debug knob)


def _build_bass():
    import concourse.mybir as mybir
    import concourse.tile as tile
    from concourse import bacc

    f32 = mybir.dt.float32
    bf16 = mybir.dt.bfloat16
    nc = bacc.Bacc("TRN2", target_bir_lowering=False, debug=False)

    # xs[b, (kh,cin), (r,w)]: kh-shifted copies premade on host; row r of
    # copy kh is padded-input row (band_start + r + kh), all 258 cols
    xs_t = nc.dram_tensor("xs", (B, 96, BAND * WP), bf16, kind="ExternalInput")
    # wc[(kh,cin), (r,kw), c]: 9 column-tile stationaries of 64 channels
    wc_t = nc.dram_tensor("wc", (96, 9, 64), bf16, kind="ExternalInput")
    # svb[(q,c), t, (s,j)]: per-pixel blend weights for ranks 1 (s=0), 2 (s=1)
    svb_t = nc.dram_tensor("svb", (128, SUPER, 2 * NBLK), bf16, kind="ExternalInput")
    # out[b, t, (q,c), (r,w)]: supertile-contiguous; host unshuffles
    out_t = nc.dram_tensor("out", (B, SUPER, 128, NBLK), bf16, kind="ExternalOutput")

    xs = xs_t.ap()
    out_ap = out_t.ap()

    with tile.TileContext(nc) as tc:
        with (
            tc.tile_pool(name="const", bufs=1) as cpool,
            tc.tile_pool(name="imcol", bufs=3) as ipool,
            tc.tile_pool(name="ps", bufs=3, space="PSUM") as pspool,
            tc.tile_pool(name="tmp", bufs=6) as tpool,
            tc.tile_pool(name="outp", bufs=8) as opool,
        ):
            # --- PE warmup: emitted first so the Tensor queue starts on it
            # at main-start (no DMA deps); releases the HAM clock gate
            # (~3.4us sustained busy) before the first real matmul arrives.
            # Two-phase warmup. Zero-fill data is INVISIBLE to the HAM
            # activity monitor (clock stayed cold through it, every run),
            # and a constant 1.5 fill power-throttled the whole body to
            # 2.0 GHz (173us) - so: 2 zero matmuls just fill the queue
            # until the wc weights land, then matmuls on wc itself
            # (random N(0,0.05) data, statistically like the real body
            # which never throttles) give HAM real toggling ~1.5us before
            # the first input chunk is consumable.
            dummy = cpool.tile([128, NBLK], bf16, tag="dummy")
            nc.vector.memset(dummy[:], 0.0)
            wps = pspool.tile([128, 2 * NBLK], f32, tag="ab", name="wps")
            for _ in range(2):
                nc.tensor.matmul(
                    wps[0:64, 0:NBLK], dummy[0:96, 0:64], dummy[0:96, :],
                    start=True, stop=True,
                )

            # --- DMA gen order = Sync program order; head-critical first.
            wc_sb = cpool.tile([96, 9, 64], bf16)

            # batch 0's band in per-supertile chunks (supertile 0 gated by
            # 198KB instead of 1.6MB)
            b0c = []
            for t in range(SUPER):
                ch = cpool.tile([96, SROWS * WP], bf16, tag=f"b0c{t}", name="ch")
                b0c.append(ch)

            def load_b0(t, eng=None):
                # t=0 goes via GpSimd's SWDGE ring, in parallel with the
                # Sync HWDGE gen stream, so the first chunk lands ~1us
                # earlier and real matmuls start at data-ready
                (eng or nc.sync).dma_start(
                    b0c[t][:], xs[0, :, SROWS * t * WP : SROWS * (t + 1) * WP]
                )

            svb_sb = cpool.tile([128, SUPER, 2 * NBLK], bf16)

            def load_svb(t):
                # per-t loads: one bulk svb DMA jammed the DVE dep chain
                # (m of supertile 2 waited on the whole 1.5MB transfer,
                # backpressured PSUM, 2.5us PE gap + HAM re-throttle)
                nc.sync.dma_start(svb_sb[:, t, :], svb_t.ap()[:, t, :])

            # DMA fabric is ~245 GB/s aggregate (all 16 engines saturate) and
            # service order = gen order, so interleave: each b0 chunk / sv_t
            # lands just ahead of the supertile that reads it.
            # first real MM gates on max(c0-rows01, wc) + ~1.2us receipt
            # each; wc gens on the idle Scalar HWDGE ring so both transfer
            # in parallel, and c0 splits so q0's matmuls gate on 99KB
            nc.scalar.dma_start(wc_sb[:], wc_t.ap())
            nc.sync.dma_start(b0c[0][:, 0 : 2 * WP], xs[0, :, 0 : 2 * WP])
            nc.sync.dma_start(
                b0c[0][:, 2 * WP : SROWS * WP], xs[0, :, 2 * WP : SROWS * WP]
            )

            # real-data warmup on the loaded weights (see above)
            wc_flat = wc_sb.rearrange("p a b -> p (a b)")
            for _ in range(2):
                nc.tensor.matmul(
                    wps[0:64, 0:NBLK], wc_sb[:, 0, :], wc_flat[:, 0:NBLK],
                    start=True, stop=True,
                )
            load_svb(0)
            load_b0(1)
            load_svb(1)
            load_b0(2)
            load_svb(2)
            load_b0(3)
            load_svb(3)
            load_b0(4)
            load_svb(4)
            load_b0(5)
            load_svb(5)
            load_b0(6)
            load_svb(6)
            load_b0(7)
            load_svb(7)

            # batch bands in 4 chunks of 8 rows (2 supertiles each; supertiles
            # never cross a chunk) so batch b+1's first supertiles only dep on
            # chunk 0, not the whole 1.6MB band transfer
            CROWS = BAND // 4

            def load_imcol(b):
                chunks = []
                for j in range(4):
                    ch = ipool.tile(
                        [96, CROWS * WP], bf16, tag="imcol", bufs=8, name="imch"
                    )
                    nc.sync.dma_start(
                        ch[:],
                        xs[b, :, CROWS * j * WP : CROWS * (j + 1) * WP],
                    )
                    chunks.append(ch)
                return chunks

            def emit_conv(iv, hl):
                """18 column-tiled conv matmuls for one supertile."""
                ab = pspool.tile([128, 2 * NBLK], f32, tag="ab", name="ab")
                c = pspool.tile([128, NBLK], f32, tag="c", bufs=2, name="c")
                # rank 1 first (free half 0), then rank 2, then rank 0 -> C:
                # C last keeps slack on its bank's WAR (bufs=2) against the
                # s-add one supertile back. (q0,q1) adjacent -> column-tile
                # pairs on disjoint PSUM partition halves.
                for r, ps in ((1, ab[:, 0:NBLK]), (2, ab[:, NBLK : 2 * NBLK]),
                              (0, c[:, :])):
                    for kw in range(3):
                        st, sp = kw == 0, kw == 2
                        for q in range(2):
                            rhs = iv[:, hl + 2 * q : hl + 2 * q + 2, kw : kw + IMG]
                            o = 64 * q
                            nc.tensor.matmul(
                                ps[o : o + 64, :],
                                wc_sb[:, 3 * r + kw, :], rhs, start=st, stop=sp,
                            )
                return ab, c

            def emit_fold(ab, c, b, t, mode="body"):
                """Blend fold. body: DVE merged-m + s, GPSIMD add, gen on
                Scalar. pen (penultimate): split m on DVE so the DVE FIFO
                drains earlier ahead of the final supertile's chain.
                final: split m on DVE, then two half-width s/add pairs on
                DVE with dma gens on Sync and Scalar in parallel."""
                # m and s in bf16: the psum-reading ops run at f32 rate
                # regardless, but the final add's operands become all-bf16
                # SBUF (DVE/GPSIMD 16-bit fast path) and tmp traffic halves;
                # extra rounding costs ~1e-3 rel err vs the 2e-2 gate
                if mode == "body":
                    # merged multiply keeps DVE busy/supertile low; split
                    # ops measured 1.91us/supertile and backpressured the
                    # PSUM ring into PE gaps
                    m = tpool.tile([128, 2 * NBLK], bf16, tag="m", name="m")
                    nc.vector.tensor_tensor(
                        m[:], ab[:], svb_sb[:, t, :], mybir.AluOpType.mult
                    )
                    m1, m2 = m[:, 0:NBLK], m[:, NBLK : 2 * NBLK]
                else:
                    # tail: split so m1 fires after the r1 matmuls (2/3 of
                    # a supertile early)
                    m1t = tpool.tile([128, NBLK], bf16, tag="m", name="m1t")
                    nc.vector.tensor_tensor(
                        m1t[:], ab[:, 0:NBLK], svb_sb[:, t, 0:NBLK],
                        mybir.AluOpType.mult,
                    )
                    m2t = tpool.tile([128, NBLK], bf16, tag="m2t", name="m2t")
                    nc.vector.tensor_tensor(
                        m2t[:], ab[:, NBLK : 2 * NBLK],
                        svb_sb[:, t, NBLK : 2 * NBLK], mybir.AluOpType.mult,
                    )
                    m1, m2 = m1t[:], m2t[:]
                if mode == "final":
                    H = NBLK // 2
                    for h, deng in enumerate((nc.sync, nc.scalar)):
                        sl = slice(H * h, H * (h + 1))
                        sh = tpool.tile([128, H], bf16, tag=f"sf{h}", name="sh")
                        nc.vector.tensor_tensor(
                            sh[:], c[:, sl], m1[:, sl], mybir.AluOpType.add
                        )
                        oh = opool.tile([128, H], bf16, tag=f"of{h}", name="oh")
                        nc.vector.tensor_tensor(
                            oh[:], sh[:], m2[:, sl], mybir.AluOpType.add
                        )
                        deng.dma_start(out_ap[b, t][:, sl], oh[:])
                    return
                s = tpool.tile([128, NBLK], bf16, tag="s", name="s")
                # s reads PSUM -> must be DVE (GPSIMD cannot access PSUM)
                nc.vector.tensor_tensor(s[:], c[:], m1, mybir.AluOpType.add)
                out_sb = opool.tile([128, NBLK], bf16, tag="out_sb", name="out_sb")
                # out-add always on GPSIMD: putting it on the DVE for the
                # tail measured WORSE (DVE FIFO is the tail bottleneck)
                nc.gpsimd.tensor_tensor(out_sb[:], s[:], m2, mybir.AluOpType.add)
                # descriptor gen on the otherwise-idle Scalar engine: keeps
                # the Sync gen stream pure input-loads (fabric priority)
                nc.scalar.dma_start(out_ap[b, t], out_sb[:])

            imcol = None
            for b in range(NB):
                imcol_nxt = load_imcol(b + 1) if b + 1 < NB else None
                for t in range(SUPER):
                    if b == 0:
                        iv, hl = b0c[t].rearrange("p (h w) -> p h w", w=WP), 0
                    else:
                        iv = imcol[t // 2].rearrange("p (h w) -> p h w", w=WP)
                        hl = SROWS * (t % 2)
                    ab, c = emit_conv(iv, hl)
                    if b == NB - 1 and t == SUPER - 1:
                        mode = "final"
                    elif b == NB - 1 and t == SUPER - 2:
                        mode = "pen"
                    else:
                        mode = "body"
                    emit_fold(ab, c, b, t, mode=mode)
                imcol = imcol_nxt
    nc.compile()
    return nc


_CACHE = {}


def _get_bass():
    if "nc" not in _CACHE:
        _CACHE["nc"] = _build_bass()
    return _CACHE["nc"]


def _prep_shards(x, conv_w, kernel_weight):
    x = np.asarray(x, dtype=_F32)
    conv_w = np.asarray(conv_w, dtype=_F32)
    kernel_weight = np.asarray(kernel_weight, dtype=_F32)

    x_pad = np.pad(x, ((0, 0), (0, 0), (1, 1), (1, 1))).astype(np_bf16)
    # wc[(kh,cin), (r,kw), c] from conv_w[(r c), cin, kh, kw]
    wc = np.ascontiguousarray(
        conv_w.reshape(RANK, C_OUT, C_IN, 3, 3)
        .transpose(3, 2, 0, 4, 1)
        .reshape(96, 9, 64)
    ).astype(np_bf16)

    in_maps = []
    for i in range(N_CORES):
        h0 = BAND * i
        # xs[b, (kh c), (r w)] = x_pad[b, c, h0 + r + kh, w]
        shard = np.ascontiguousarray(
            np.stack(
                [x_pad[:, :, h0 + kh : h0 + kh + BAND, :] for kh in range(3)],
                axis=1,
            )
        ).reshape(B, 96, BAND * WP)
        band = kernel_weight[:, h0 : h0 + BAND, :]          # [2, 32, 256]
        # svb[64q+c, t, (s,j)] = band[s, 4t+2q+(j//256), j%256]
        tmp = band.reshape(2, SUPER, 2, NBLK)               # [s, t, q, j]
        svb = np.broadcast_to(
            tmp.transpose(2, 1, 0, 3)[:, None],             # [q, 1, t, s, j]
            (2, C_OUT, SUPER, 2, NBLK),
        ).reshape(128, SUPER, 2 * NBLK)
        svb = np.ascontiguousarray(svb).astype(np_bf16)
        in_maps.append({"xs": shard, "wc": wc, "svb": svb})
    return in_maps


def run(inputs, trace=False):
    """Run the sharded bass kernel; returns (out_full, BassKernelResults)."""
    from concourse.bass_utils import run_bass_kernel_spmd

    in_maps = _prep_shards(**inputs)
    nc = _get_bass()
    res = run_bass_kernel_spmd(
        nc, in_maps, core_ids=list(range(N_CORES)), trace=trace
    )
    out = np.empty((B, C_OUT, IMG, IMG), dtype=_F32)
    for i in range(N_CORES):
        # res: [B, SUPER, (q c), (r w)] -> [B, c, (t q r), w]
        band = (
            np.asarray(res.results[i]["out"], dtype=_F32)
            .reshape(B, SUPER, 2, C_OUT, 2, IMG)
            .transpose(0, 3, 1, 2, 4, 5)
            .reshape(B, C_OUT, BAND, IMG)
        )
        out[:, :, BAND * i : BAND * (i + 1), :] = band
    return out, res


def kernel(x, conv_w, kernel_weight):
    out, _ = run({"x": x, "conv_w": conv_w, "kernel_weight": kernel_weight})
    return out


# revision 34
# speedup vs baseline: 1.2068x; 1.0098x over previous
"""Trainium2 Bass kernel for nn_LRSVConv (low-rank spatially-varying conv).

Computes, for full inputs
    x            [8, 32, 256, 256]  f32
    conv_w       [192, 32, 3, 3]    f32   (192 = RANK(3) * C_OUT(64))
    kernel_weight[2, 256, 256]      f32
the reference:
    y   = conv2d(x, conv_w, stride 1, pad 1)      # [8, 192, 256, 256]
    y   = y.reshape(8, 3, 64, 256, 256)
    out = y[:,0] + kw[0]*y[:,1] + kw[1]*y[:,2]    # [8, 64, 256, 256]

Strategy: spatial (H) sharding across 8 cores - each core computes a band of
32 output rows for ALL batches, so the per-pixel blend weights (which are
batch-independent) are loaded once per core and reused 8x.

Final design (v1 baseline measured 150.2-150.5us; this version measures
142.5-145.4us across runs, variance is HAM clock-gate phase luck). The MM
body runs dense at the 216ns/pair-slot warm roofline - the 9-slot/supertile
decomposition's floor is 124.7us and is provably minimal for bf16 direct
conv (ceil(288/128)=3 K-steps x 1.5 M-pairs x 2 px-blocks; fp8 fails the
accuracy gate, tap-baked K=128 layouts exceed the ~245 GB/s measured DMA
fabric). So v2+ attacked the head (was 12.5us to first MM + ~3us cold-clock
penalty) and tail (was ~8us serial fold+DMA after the last MM):
  - bf16 inputs/weights (host-converted; f32 PSUM accumulation).
  - PE column tiling: per supertile (4 rows = 2 blocks q of 512 px),
    9 pair-slots of [96,64,512] matmuls: rank 1 -> ab[:, 0:512], rank 2 ->
    ab[:, 512:1024], rank 0 -> C (last: its bufs=2 bank has a WAR on the
    s-add one supertile back); (q0,q1) adjacent -> concurrent column tiles.
  - kh-baked input layout is premade IN DRAM by the host (xs[b, (kh c),
    band rows]): 1 dma_start per band chunk, no on-chip shuffling. Bands
    load in 4 chunks of 8 rows (dependency granularity: batch b+1's first
    supertiles only dep on chunk 0 - DMA fabric saturates at ~245 GB/s
    aggregate and a full-band transfer arrives too late at batch switch).
  - batch 0's band in 8 per-supertile chunks; c0 gen precedes the wc gen
    (first real MM gates on max of both + ~1.2us DMA completion-receipt
    latency -> first real MM at ~10us). Warmup matmuls fill the pre-data
    window: 2 on a zeroed tile (zero data is invisible to the HAM activity
    monitor, but they keep the queue primed), then 2 on the loaded wc
    weights (real random data, so HAM sees toggling ~1.5us before the
    first input chunk is consumable). Starting real MMs ASAP beats idling
    for the clock - cold slots still make progress at half rate.
    NOTE: runs land in occasional ~174us windows where the chip is
    power-throttled to ~2.0 GHz from the very first (zero-data) matmul -
    environmental, not data-dependent.
  - fold on the non-PE engines (GPSIMD cannot read PSUM, so psum-reading
    ops live on DVE; DVE op cost ~ free-dim size only):
      DVE:    m = ab * sv (merged [128,1024], bf16 out), s = C + m1
      GPSIMD: out_sb = s + m2 (all-bf16 operands, 1.14us)
    bf16 m/s costs ~1.6e-3 extra rel err (5.8e-3 total vs the 2e-2 gate).
  - output written as [b, t, (q c), (r w)] so each supertile is ONE
    contiguous [128,512] dma_start; descriptor gen on the otherwise-idle
    Scalar engine (gen is ~0.6us serial per issuing engine; Sync's stream
    stays pure input-loads = fabric priority). Host un-shuffles at gather.
  - tail: the last two supertiles split m into m1/m2 (m1 fires 2/3 of a
    supertile early); the final supertile folds in two half-width pieces
    with dma gens on Sync and Scalar in parallel. ~3.3us from last matmul
    to last byte, then a fixed ~9us runtime teardown ceremony.
"""

import os

import numpy as np
from ml_dtypes import bfloat16 as np_bf16

B, C_IN, C_OUT, RANK, IMG = 8, 32, 64, 3, 256
N_CORES = 8
BAND = IMG // N_CORES          # 32 output rows per core
WP = IMG + 2                   # padded width 258
SUPER = 8                      # supertiles per (batch, band): 4 rows each
SROWS = BAND // SUPER          # 4 image rows per supertile
NBLK = 512                     # pixels per matmul block (2 image rows)
# warmup: 2 zero-data queue-filler matmuls + 2 real-data (wc) matmuls

_F32 = np.float32

NB = int(os.environ.get("KERNEL_NB", str(B)))  # batches to process (debug knob)


def _build_bass():
    import concourse.mybir as mybir
    import concourse.tile as tile
    from concourse import bacc

    f32 = mybir.dt.float32
    bf16 = mybir.dt.bfloat16
    nc = bacc.Bacc("TRN2", target_bir_lowering=False, debug=False)

    # xs[b, (kh,cin), (r,w)]: kh-shifted copies premade on host; row r of
    # copy kh is padded-input row (band_start + r + kh), all 258 cols
    xs_t = nc.dram_tensor("xs", (B, 96, BAND * WP), bf16, kind="ExternalInput")
    # wc[(kh,cin), (r,kw), c]: 9 column-tile stationaries of 64 channels
    wc_t = nc.dram_tensor("wc", (96, 9, 64), bf16, kind="ExternalInput")
    # svb[(q,c), t, (s,j)]: per-pixel blend weights for ranks 1 (s=0), 2 (s=1)
    svb_t = nc.dram_tensor("svb", (128, SUPER, 2 * NBLK), bf16, kind="ExternalInput")
    # out[b, t, (q,c), (r,w)]: supertile-contiguous; host unshuffles
    out_t = nc.dram_tensor("out", (B, SUPER, 128, NBLK), bf16, kind="ExternalOutput")

    xs = xs_t.ap()
    out_ap = out_t.ap()

    with tile.TileContext(nc) as tc:
        with (
            tc.tile_pool(name="const", bufs=1) as cpool,
            tc.tile_pool(name="imcol", bufs=3) as ipool,
            tc.tile_pool(name="ps", bufs=3, space="PSUM") as pspool,
            tc.tile_pool(name="tmp", bufs=6) as tpool,
            tc.tile_pool(name="outp", bufs=8) as opool,
        ):
            # --- PE warmup: emitted first so the Tensor queue starts on it
            # at main-start (no DMA deps); releases the HAM clock gate
            # (~3.4us sustained busy) before the first real matmul arrives.
            # Two-phase warmup. Zero-fill data is INVISIBLE to the HAM
            # activity monitor (clock stayed cold through it, every run),
            # and a constant 1.5 fill power-throttled the whole body to
            # 2.0 GHz (173us) - so: 2 zero matmuls just fill the queue
            # until the wc weights land, then matmuls on wc itself
            # (random N(0,0.05) data, statistically like the real body
            # which never throttles) give HAM real toggling ~1.5us before
            # the first input chunk is consumable.
            dummy = cpool.tile([128, NBLK], bf16, tag="dummy")
            nc.vector.memset(dummy[:], 0.0)
            wps = pspool.tile([128, 2 * NBLK], f32, tag="ab", name="wps")
            for _ in range(2):
                nc.tensor.matmul(
                    wps[0:64, 0:NBLK], dummy[0:96, 0:64], dummy[0:96, :],
                    start=True, stop=True,
                )

            # --- DMA gen order = Sync program order; head-critical first.
            wc_sb = cpool.tile([96, 9, 64], bf16)

            # batch 0's band in per-supertile chunks (supertile 0 gated by
            # 198KB instead of 1.6MB)
            b0c = []
            for t in range(SUPER):
                ch = cpool.tile([96, SROWS * WP], bf16, tag=f"b0c{t}", name="ch")
                b0c.append(ch)

            def load_b0(t, eng=None):
                # t=0 goes via GpSimd's SWDGE ring, in parallel with the
                # Sync HWDGE gen stream, so the first chunk lands ~1us
                # earlier and real matmuls start at data-ready
                (eng or nc.sync).dma_start(
                    b0c[t][:], xs[0, :, SROWS * t * WP : SROWS * (t + 1) * WP]
                )

            svb_sb = cpool.tile([128, SUPER, 2 * NBLK], bf16)

            def load_svb(t):
                # per-t loads: one bulk svb DMA jammed the DVE dep chain
                # (m of supertile 2 waited on the whole 1.5MB transfer,
                # backpressured PSUM, 2.5us PE gap + HAM re-throttle)
                nc.sync.dma_start(svb_sb[:, t, :], svb_t.ap()[:, t, :])

            # DMA fabric is ~245 GB/s aggregate (all 16 engines saturate) and
            # service order = gen order, so interleave: each b0 chunk / sv_t
            # lands just ahead of the supertile that reads it.
            # first real MM gates on max(c0-rows01, wc) + ~1.2us receipt
            # each; wc gens on the idle Scalar HWDGE ring so both transfer
            # in parallel, and c0 splits so q0's matmuls gate on 99KB.
            # wc itself splits 33KB/77KB: the tiny piece A (stationaries
            # 0-2) lands ~1.5us earlier and reliably, so the real-data
            # warmup (which only reads piece A) un-throttles the clock
            # ahead of the input data regardless of fabric jitter.
            nc.scalar.dma_start(wc_sb[:, 0:3, :], wc_t.ap()[:, 0:3, :])
            nc.scalar.dma_start(wc_sb[:, 3:9, :], wc_t.ap()[:, 3:9, :])
            nc.sync.dma_start(b0c[0][:, 0 : 2 * WP], xs[0, :, 0 : 2 * WP])
            nc.sync.dma_start(
                b0c[0][:, 2 * WP : SROWS * WP], xs[0, :, 2 * WP : SROWS * WP]
            )

            # real-data warmup on weight piece A (see above)
            wc_flat = wc_sb.rearrange("p a b -> p (a b)")
            for _ in range(4):
                nc.tensor.matmul(
                    wps[0:64, 0:192], wc_sb[:, 0, :], wc_flat[:, 0:192],
                    start=True, stop=True,
                )
            load_svb(0)
            load_b0(1)
            load_svb(1)
            load_b0(2)
            load_svb(2)
            load_b0(3)
            load_svb(3)
            load_b0(4)
            load_svb(4)
            load_b0(5)
            load_svb(5)
            load_b0(6)
            load_svb(6)
            load_b0(7)
            load_svb(7)

            # batch bands in 4 chunks of 8 rows (2 supertiles each; supertiles
            # never cross a chunk) so batch b+1's first supertiles only dep on
            # chunk 0, not the whole 1.6MB band transfer
            CROWS = BAND // 4

            def load_imcol(b):
                chunks = []
                for j in range(4):
                    ch = ipool.tile(
                        [96, CROWS * WP], bf16, tag="imcol", bufs=8, name="imch"
                    )
                    nc.sync.dma_start(
                        ch[:],
                        xs[b, :, CROWS * j * WP : CROWS * (j + 1) * WP],
                    )
                    chunks.append(ch)
                return chunks

            def emit_conv(iv, hl):
                """18 column-tiled conv matmuls for one supertile."""
                ab = pspool.tile([128, 2 * NBLK], f32, tag="ab", name="ab")
                c = pspool.tile([128, NBLK], f32, tag="c", bufs=2, name="c")
                # rank 1 first (free half 0), then rank 2, then rank 0 -> C:
                # C last keeps slack on its bank's WAR (bufs=2) against the
                # s-add one supertile back. (q0,q1) adjacent -> column-tile
                # pairs on disjoint PSUM partition halves.
                for r, ps in ((1, ab[:, 0:NBLK]), (2, ab[:, NBLK : 2 * NBLK]),
                              (0, c[:, :])):
                    for kw in range(3):
                        st, sp = kw == 0, kw == 2
                        for q in range(2):
                            rhs = iv[:, hl + 2 * q : hl + 2 * q + 2, kw : kw + IMG]
                            o = 64 * q
                            nc.tensor.matmul(
                                ps[o : o + 64, :],
                                wc_sb[:, 3 * r + kw, :], rhs, start=st, stop=sp,
                            )
                return ab, c

            def emit_fold(ab, c, b, t, mode="body"):
                """Blend fold. body: DVE merged-m + s, GPSIMD add, gen on
                Scalar. pen (penultimate): split m on DVE so the DVE FIFO
                drains earlier ahead of the final supertile's chain.
                final: split m on DVE, then two half-width s/add pairs on
                DVE with dma gens on Sync and Scalar in parallel."""
                # m and s in bf16: the psum-reading ops run at f32 rate
                # regardless, but the final add's operands become all-bf16
                # SBUF (DVE/GPSIMD 16-bit fast path) and tmp traffic halves;
                # extra rounding costs ~1e-3 rel err vs the 2e-2 gate
                if mode == "body":
                    # merged multiply keeps DVE busy/supertile low; split
                    # ops measured 1.91us/supertile and backpressured the
                    # PSUM ring into PE gaps
                    m = tpool.tile([128, 2 * NBLK], bf16, tag="m", name="m")
                    nc.vector.tensor_tensor(
                        m[:], ab[:], svb_sb[:, t, :], mybir.AluOpType.mult
                    )
                    m1, m2 = m[:, 0:NBLK], m[:, NBLK : 2 * NBLK]
                else:
                    # tail: split so m1 fires after the r1 matmuls (2/3 of
                    # a supertile early)
                    m1t = tpool.tile([128, NBLK], bf16, tag="m", name="m1t")
                    nc.vector.tensor_tensor(
                        m1t[:], ab[:, 0:NBLK], svb_sb[:, t, 0:NBLK],
                        mybir.AluOpType.mult,
                    )
                    m2t = tpool.tile([128, NBLK], bf16, tag="m2t", name="m2t")
                    nc.vector.tensor_tensor(
                        m2t[:], ab[:, NBLK : 2 * NBLK],
                        svb_sb[:, t, NBLK : 2 * NBLK], mybir.AluOpType.mult,
                    )
                    m1, m2 = m1t[:], m2t[:]
                if mode == "final":
                    H = NBLK // 2
                    for h, deng in enumerate((nc.sync, nc.scalar)):
                        sl = slice(H * h, H * (h + 1))
                        sh = tpool.tile([128, H], bf16, tag=f"sf{h}", name="sh")
                        nc.vector.tensor_tensor(
                            sh[:], c[:, sl], m1[:, sl], mybir.AluOpType.add
                        )
                        oh = opool.tile([128, H], bf16, tag=f"of{h}", name="oh")
                        nc.vector.tensor_tensor(
                            oh[:], sh[:], m2[:, sl], mybir.AluOpType.add
                        )
                        deng.dma_start(out_ap[b, t][:, sl], oh[:])
                    return
                s = tpool.tile([128, NBLK], bf16, tag="s", name="s")
                # s reads PSUM -> must be DVE (GPSIMD cannot access PSUM)
                nc.vector.tensor_tensor(s[:], c[:], m1, mybir.AluOpType.add)
                out_sb = opool.tile([128, NBLK], bf16, tag="out_sb", name="out_sb")
                # out-add always on GPSIMD: putting it on the DVE for the
                # tail measured WORSE (DVE FIFO is the tail bottleneck)
                nc.gpsimd.tensor_tensor(out_sb[:], s[:], m2, mybir.AluOpType.add)
                # descriptor gen on the otherwise-idle Scalar engine: keeps
                # the Sync gen stream pure input-loads (fabric priority)
                nc.scalar.dma_start(out_ap[b, t], out_sb[:])

            imcol = None
            for b in range(NB):
                imcol_nxt = load_imcol(b + 1) if b + 1 < NB else None
                for t in range(SUPER):
                    if b == 0:
                        iv, hl = b0c[t].rearrange("p (h w) -> p h w", w=WP), 0
                    else:
                        iv = imcol[t // 2].rearrange("p (h w) -> p h w", w=WP)
                        hl = SROWS * (t % 2)
                    ab, c = emit_conv(iv, hl)
                    if b == NB - 1 and t == SUPER - 1:
                        mode = "final"
                    elif b == NB - 1 and t == SUPER - 2:
                        mode = "pen"
                    else:
                        mode = "body"
                    emit_fold(ab, c, b, t, mode=mode)
                imcol = imcol_nxt
    nc.compile()
    return nc


_CACHE = {}


def _get_bass():
    if "nc" not in _CACHE:
        _CACHE["nc"] = _build_bass()
    return _CACHE["nc"]


def _prep_shards(x, conv_w, kernel_weight):
    x = np.asarray(x, dtype=_F32)
    conv_w = np.asarray(conv_w, dtype=_F32)
    kernel_weight = np.asarray(kernel_weight, dtype=_F32)

    x_pad = np.pad(x, ((0, 0), (0, 0), (1, 1), (1, 1))).astype(np_bf16)
    # wc[(kh,cin), (r,kw), c] from conv_w[(r c), cin, kh, kw]
    wc = np.ascontiguousarray(
        conv_w.reshape(RANK, C_OUT, C_IN, 3, 3)
        .transpose(3, 2, 0, 4, 1)
        .reshape(96, 9, 64)
    ).astype(np_bf16)

    in_maps = []
    for i in range(N_CORES):
        h0 = BAND * i
        # xs[b, (kh c), (r w)] = x_pad[b, c, h0 + r + kh, w]
        shard = np.ascontiguousarray(
            np.stack(
                [x_pad[:, :, h0 + kh : h0 + kh + BAND, :] for kh in range(3)],
                axis=1,
            )
        ).reshape(B, 96, BAND * WP)
        band = kernel_weight[:, h0 : h0 + BAND, :]          # [2, 32, 256]
        # svb[64q+c, t, (s,j)] = band[s, 4t+2q+(j//256), j%256]
        tmp = band.reshape(2, SUPER, 2, NBLK)               # [s, t, q, j]
        svb = np.broadcast_to(
            tmp.transpose(2, 1, 0, 3)[:, None],             # [q, 1, t, s, j]
            (2, C_OUT, SUPER, 2, NBLK),
        ).reshape(128, SUPER, 2 * NBLK)
        svb = np.ascontiguousarray(svb).astype(np_bf16)
        in_maps.append({"xs": shard, "wc": wc, "svb": svb})
    return in_maps


def run(inputs, trace=False):
    """Run the sharded bass kernel; returns (out_full, BassKernelResults)."""
    from concourse.bass_utils import run_bass_kernel_spmd

    in_maps = _prep_shards(**inputs)
    nc = _get_bass()
    res = run_bass_kernel_spmd(
        nc, in_maps, core_ids=list(range(N_CORES)), trace=trace
    )
    out = np.empty((B, C_OUT, IMG, IMG), dtype=_F32)
    for i in range(N_CORES):
        # res: [B, SUPER, (q c), (r w)] -> [B, c, (t q r), w]
        band = (
            np.asarray(res.results[i]["out"], dtype=_F32)
            .reshape(B, SUPER, 2, C_OUT, 2, IMG)
            .transpose(0, 3, 1, 2, 4, 5)
            .reshape(B, C_OUT, BAND, IMG)
        )
        out[:, :, BAND * i : BAND * (i + 1), :] = band
    return out, res


def kernel(x, conv_w, kernel_weight):
    out, _ = run({"x": x, "conv_w": conv_w, "kernel_weight": kernel_weight})
    return out
